# revision 24
# baseline (speedup 1.0000x reference)
"""Causal multi-head attention on 8 Trainium2 NeuronCores.

Problem: x[2,4096,512], W_q/W_k/W_v/W_proj[512,512], b_proj[512]
  q,k,v = x @ W.T split into 8 heads of 64; causal softmax(q k^T / 8) v;
  out = attn @ W_proj.T + b_proj.

Sharding: 16 (batch, head) pairs over 8 cores -> each core gets one batch
and a pair of adjacent heads (128 of the 512 hidden dims).  The output
projection is computed per-core against the matching 128-row slice of
W_proj^T, giving a partial [4096, 512] output per core; the host sums the
4 partials per batch and adds the bias.

Per-core design (ScalarE exp is the roofline: ~17.6M exps ~= 115 us):
  xT    [512, 4096] streamed per 512-col chunk (prefetch 2 chunks ahead)
  qT/kT [128, 4096] rows 0-63 head0, 64-127 head1 (both heads' score
        matmuls are contraction-64 row-tiles at base partitions 0/64 ->
        they can run concurrently on the PE when interleaved)
  scores per (h, kbt): [128 k, 1024 q] PSUM -> exp on ScalarE -> ex SBUF;
        diagonal blocks restrict the q range to the causally valid part
        (capped so the fp32r free dim stays >= 256)
  causal mask: gpsimd affine_select zeroes invalid (q < k) entries of the
        diagonal blocks in place (no mask tensors, off the DVE)
  attnV: v65 (v plus a ones row) x ex accumulates [65, 512] per head; row
        64 is the softmax denominator
  normalization is folded into the *output projection* evacuation: each
        head is projected separately (contraction-64 row-tile pair) and the
        PSUM->SBUF evac applies the per-q reciprocal denominator:
        ot = pp0 * r0[q] + pp1 * r1[q]  (tensor_scalar + scalar_tensor_tensor)
  one flat software pipeline over all (chunk, kbt) units: scores/exp of
        unit i overlap attnV of unit i-1 across chunk boundaries, and all
        deferred work (x DMA, q/k/v projections two chunks ahead, the
        normalize/project/store of finished chunks) is queued with
        deadlines and drained in SMALL pieces, one per unit, into the
        PE/DVE slack under the continuous ScalarE exp stream.  The fine
        granularity matters on hardware: engine queues are shallow (8) and
        strictly FIFO, so any burst of one engine's work head-of-line
        blocks dependents and stalls the exp stream (sim won't show this).
  PSUM budget (8 banks): 2x scores [128,1024] (4) + 2x acc [65,512] (2) +
        2 matmul staging banks shared by projections / output pairs /
        transposes (2).
"""

import numpy as np

B, S, D, H = 2, 4096, 512, 8
DH = 64
QCHUNK = 512
SCALE = 1.0 / np.sqrt(DH)

_CACHE = {}


def _build(s=S, repeats=1):
    from contextlib import ExitStack

    import concourse.mybir as mybir
    import concourse.tile as tile
    from concourse import bacc
    from concourse.masks import make_identity

    f32 = mybir.dt.float32
    f32r = mybir.dt.float32r
    EXP = mybir.ActivationFunctionType.Exp
    GE = mybir.AluOpType.is_ge
    MULT = mybir.AluOpType.mult
    ADD = mybir.AluOpType.add

    nqc = s // QCHUNK      # q chunks
    nkb_all = s // 128     # k blocks
    ndc = D // 128         # D chunks (contraction for projections)
    kbc = QCHUNK // 128    # k blocks per q chunk

    nc = bacc.Bacc("TRN2")
    xT_d = nc.dram_tensor("xT", [D, s], f32r, kind="ExternalInput")
    wqT_d = nc.dram_tensor("wqT", [D, 128], f32r, kind="ExternalInput")
    wkT_d = nc.dram_tensor("wkT", [D, 128], f32r, kind="ExternalInput")
    wvT_d = nc.dram_tensor("wvT", [D, 128], f32r, kind="ExternalInput")
    wpT_d = nc.dram_tensor("wpT", [128, D], f32r, kind="ExternalInput")
    ones_d = nc.dram_tensor("ones_in", [128, 64], f32r, kind="ExternalInput")
    out_d = nc.dram_tensor("out_p", [s, D], f32, kind="ExternalOutput")

    with ExitStack() as ctx:
        tc = ctx.enter_context(tile.TileContext(nc))
        consts = ctx.enter_context(tc.tile_pool(name="consts", bufs=1))
        big = ctx.enter_context(tc.tile_pool(name="big", bufs=1))
        expool = ctx.enter_context(tc.tile_pool(name="expool", bufs=3))
        outpool = ctx.enter_context(tc.tile_pool(name="outpool", bufs=3))
        smallp = ctx.enter_context(tc.tile_pool(name="smallp", bufs=3))
        scps = ctx.enter_context(tc.tile_pool(name="scps", bufs=1, space="PSUM"))
        accps = ctx.enter_context(tc.tile_pool(name="accps", bufs=1, space="PSUM"))
        mmps = ctx.enter_context(tc.tile_pool(name="mmps", bufs=1, space="PSUM"))

        # ---- persistent SBUF ----
        xT = [big.tile([128, s], f32r, name=f"xT{c}", tag=f"xT{c}") for c in range(ndc)]
        qT = big.tile([128, s], f32r, name="qT", tag="qT")
        kT = big.tile([128, s], f32r, name="kT", tag="kT")
        v65 = [big.tile([128, 65 * nkb_all], f32r, name=f"v65_{h}", tag=f"v65_{h}")
               for h in range(2)]
        attnT = big.tile([128, s], f32r, name="attnT", tag="attnT")
        rec = [big.tile([128, nkb_all], f32, name=f"rec{h}", tag=f"rec{h}")
               for h in range(2)]
        wq = consts.tile([128, D], f32r, name="wq", tag="wq")
        wk = consts.tile([128, D], f32r, name="wk", tag="wk")
        wv = consts.tile([128, D], f32r, name="wv", tag="wv")
        wp = consts.tile([128, D], f32r, name="wp", tag="wp")
        ident = consts.tile([128, 128], f32, name="ident", tag="ident")

        env = dict(
            nc=nc, tc=tc, mybir=mybir, f32=f32, f32r=f32r, EXP=EXP, GE=GE,
            MULT=MULT, ADD=ADD, s=s, nqc=nqc, nkb_all=nkb_all, ndc=ndc,
            kbc=kbc, xT_d=xT_d, wqT_d=wqT_d, wkT_d=wkT_d, wvT_d=wvT_d,
            wpT_d=wpT_d, ones_d=ones_d, out_d=out_d, xT=xT, qT=qT, kT=kT,
            v65=v65, attnT=attnT, rec=rec, wq=wq, wk=wk, wv=wv, wp=wp,
            ident=ident, consts=consts, big=big, expool=expool,
            outpool=outpool, smallp=smallp, scps=scps, accps=accps,
            mmps=mmps,
        )

        # ---- one-time constants ----
        # weights go on the Activation HWDGE queue so they overlap the xT
        # chunk DMAs on the sync queue; preload the exp table while idle
        for w_sb, w_d in ((wq, wqT_d), (wk, wkT_d), (wv, wvT_d)):
            for c in range(ndc):
                nc.scalar.dma_start(out=w_sb[:, c * 128:(c + 1) * 128],
                                 in_=w_d[c * 128:(c + 1) * 128, :])
        nc.scalar.dma_start(out=wp, in_=wpT_d.ap())
        for h in range(2):
            ones_ap = v65[h].rearrange("p (k c) -> p k c", c=65)[:, :, 64]
            nc.scalar.dma_start(out=ones_ap, in_=ones_d[:, 0:nkb_all])
        make_identity(nc, ident)
        nc.scalar.activation(ident[0:1, 0:1], ident[0:1, 0:1], EXP, scale=0.0)

        for rep in range(repeats):
            _emit_body(env, rep)

    nc.compile()
    return nc


def _dma_x(env, qc):
    nc, xT, xT_d, ndc = env["nc"], env["xT"], env["xT_d"], env["ndc"]
    qs = slice(qc * QCHUNK, (qc + 1) * QCHUNK)
    for c in range(ndc):
        nc.sync.dma_start(out=xT[c][:, qs], in_=xT_d[c * 128:(c + 1) * 128, qs])


def _proj_qk(env, qc):
    nc, f32, ndc = env["nc"], env["f32"], env["ndc"]
    mmps, xT, qT, kT, wq, wk = (env["mmps"], env["xT"], env["qT"], env["kT"],
                                env["wq"], env["wk"])
    qs = slice(qc * QCHUNK, (qc + 1) * QCHUNK)
    for i, (w_sb, dst) in enumerate(((wq, qT), (wk, kT))):
        ps = mmps.tile([128, QCHUNK], f32, name=f"proj{i}_{qc}", tag=f"m{i}")
        for c in range(ndc):
            nc.tensor.matmul(ps, lhsT=w_sb[:, c * 128:(c + 1) * 128],
                             rhs=xT[c][:, qs],
                             start=(c == 0), stop=(c == ndc - 1))
        nc.vector.tensor_copy(dst[:, qs], ps)


def _proj_v(env, qc, half=None):
    nc, f32, ndc, kbc = env["nc"], env["f32"], env["ndc"], env["kbc"]
    mmps, xT, wv, v65 = env["mmps"], env["xT"], env["wv"], env["v65"]
    js = range(kbc) if half is None else range(half * 2, half * 2 + 2)
    for j in js:
        kb = qc * kbc + j
        vp = mmps.tile([128, 128], f32, name=f"vp_{kb}", tag=f"m{j % 2}")
        for c in range(ndc):
            nc.tensor.matmul(vp, lhsT=xT[c][:, kb * 128:(kb + 1) * 128],
                             rhs=wv[:, c * 128:(c + 1) * 128],
                             start=(c == 0), stop=(c == ndc - 1))
        for h in range(2):
            nc.vector.tensor_copy(v65[h][:, kb * 65:kb * 65 + 64],
                                  vp[:, h * 64:(h + 1) * 64])


def _q0(env, qc, kb):
    """Valid-q start col for block kb within the chunk (0 if off-diagonal).

    Capped at 256 so the fp32r matmul free dim stays >= 256 (below that the
    PE runs the slow 4-cycle/row path).
    """
    r = kb - qc * env["kbc"]
    return min(128 * r, 256) if r >= 0 else 0


def _scores(env, qc, kbt, sc):
    """4 score MMs (heads interleaved -> concurrent 64-row PE tiles), 2 exps."""
    nc, f32, f32r, EXP = env["nc"], env["f32"], env["f32r"], env["EXP"]
    expool, qT, kT = env["expool"], env["qT"], env["kT"]
    qlo = qc * QCHUNK
    for h in range(2):
        sc[h] = env["scps"].tile([128, 1024], f32, name=f"sc{h}_{qc}_{kbt}",
                                 tag=f"sc{h}")
    q0s = [_q0(env, qc, kbt * 2 + j) for j in range(2)]
    for j in range(2):
        kb = kbt * 2 + j
        for h in range(2):
            hsl = slice(h * 64, (h + 1) * 64)
            nc.tensor.matmul(sc[h][:, j * 512 + q0s[j]:(j + 1) * 512],
                             lhsT=kT[hsl, kb * 128:(kb + 1) * 128],
                             rhs=qT[hsl, qlo + q0s[j]:qlo + QCHUNK],
                             start=True, stop=True)
    ex = [None, None]
    for h in range(2):
        ex[h] = expool.tile([128, 1024], f32r, name=f"ex{h}_{qc}_{kbt}",
                            tag=f"ex{h}")
        if q0s[0] == 0:
            nc.scalar.activation(ex[h], sc[h], EXP, scale=float(SCALE))
        else:  # both blocks restricted: two exact-range exps.  The exp only
            # has to cover the causally valid cols (128*r uncapped) -- the
            # affine_select fill below replaces everything before that, so
            # the strip the matmul wrote just to keep its free dim >= 256
            # needs no exp.
            for j in range(2):
                kb = kbt * 2 + j
                q0e = 128 * (kb - qc * env["kbc"])
                sl = slice(j * 512 + q0e, (j + 1) * 512)
                nc.scalar.activation(ex[h][:, sl], sc[h][:, sl], EXP,
                                     scale=float(SCALE))
    # causal mask on diagonal blocks: zero where q < k (gpsimd, in place)
    for j in range(2):
        kb = kbt * 2 + j
        r = kb - qc * env["kbc"]
        if r >= 0:
            w = 512 - q0s[j]
            for h in range(2):
                nc.gpsimd.affine_select(
                    out=ex[h][:, j * 512 + q0s[j]:(j + 1) * 512],
                    in_=ex[h][:, j * 512 + q0s[j]:(j + 1) * 512],
                    compare_op=env["GE"], fill=0.0,
                    base=-(128 * r - q0s[j]), channel_multiplier=-1,
                    pattern=[[1, w]])
    return ex


def _attnv(env, qc, kbt, acc, ex, nkb):
    nc, v65 = env["nc"], env["v65"]
    for j in range(2):
        kb = kbt * 2 + j
        q0 = _q0(env, qc, kb)
        for h in range(2):
            nc.tensor.matmul(acc[h][:, q0:QCHUNK],
                             lhsT=v65[h][:, kb * 65:(kb + 1) * 65],
                             rhs=ex[h][:, j * 512 + q0:(j + 1) * 512],
                             start=(kb == 0), stop=(kb == nkb - 1))


def _acc_evac(env, qc, acc, sums):
    """sums + attnT copies right after the last attnV (frees acc banks).

    Both sums copies go first so the denominator transpose chains (DMA ->
    PE transpose -> reciprocal) for the two heads launch as early as
    possible."""
    nc, smallp, attnT, f32 = env["nc"], env["smallp"], env["attnT"], env["f32"]
    qs = slice(qc * QCHUNK, (qc + 1) * QCHUNK)
    for h in range(2):
        sums[h] = smallp.tile([1, QCHUNK], f32, name=f"sums{h}_{qc}",
                              tag=f"su{h}")
        nc.vector.tensor_copy(sums[h], acc[h][64:65, :])
    for h in range(2):
        nc.vector.tensor_copy(attnT[h * 64:(h + 1) * 64, qs], acc[h][0:64, :])


def _norm_out(env, qc, sums=None, blocks=(), last=False):
    """Deferred: transpose denominators, reciprocal, project + fused scale.

    Called once with sums (the reciprocal prep) and then per block pair so
    the DVE evacuation load spreads across the kbt pipeline slots.
    For the final chunk (last=True) the ScalarE is idle, so it takes the
    pp0 half of each evacuation (Copy with per-partition scale) and the pp
    pairs rotate through the freed score PSUM banks as well, shortening
    the serial tail."""
    nc, f32, f32r, kbc = env["nc"], env["f32"], env["f32r"], env["kbc"]
    mmps, smallp, outpool = env["mmps"], env["smallp"], env["outpool"]
    attnT, rec, wp, ident, out_d = (env["attnT"], env["rec"], env["wp"],
                                    env["ident"], env["out_d"])
    MULT, ADD = env["MULT"], env["ADD"]
    COPY = env["mybir"].ActivationFunctionType.Copy
    if sums is not None:
        for h in range(2):
            rf2 = smallp.tile([kbc, 128], f32, name=f"rf2{h}_{qc}", tag=f"rf{h}")
            nc.sync.dma_start(out=rf2,
                              in_=sums[h].rearrange("o (c p) -> o c p", p=128))
            rfp = mmps.tile([128, kbc], f32, name=f"rfp{h}_{qc}", tag=f"m{h}")
            nc.tensor.transpose(rfp, rf2, ident[0:kbc, 0:kbc])
            nc.vector.reciprocal(rec[h][:, qc * kbc:(qc + 1) * kbc], rfp)
        return
    for j in blocks:
        qb = qc * kbc + j
        pp = [None, None]
        for h in range(2):
            hsl = slice(h * 64, (h + 1) * 64)
            if last and j % 2 == 0:
                pool, tag = env["scps"], f"sc{h}"
            else:
                pool, tag = mmps, f"m{h}"
            pp[h] = pool.tile([128, D], f32, name=f"pp{h}_{qb}", tag=tag)
            nc.tensor.matmul(pp[h], lhsT=attnT[hsl, qb * 128:(qb + 1) * 128],
                             rhs=wp[hsl, :], start=True, stop=True)
        ot = outpool.tile([128, D], f32, name=f"ot_{qb}", tag="ot")
        if last:
            nc.scalar.activation(ot, pp[0], COPY, scale=rec[0][:, qb:qb + 1])
        else:
            nc.vector.tensor_scalar_mul(ot, pp[0], rec[0][:, qb:qb + 1])
        nc.vector.scalar_tensor_tensor(ot, in0=pp[1], scalar=rec[1][:, qb:qb + 1],
                                       in1=ot, op0=MULT, op1=ADD)
        nc.sync.dma_start(out=out_d[qb * 128:(qb + 1) * 128, :], in_=ot)


def _emit_body(env, rep):
    """Flat software pipeline over all (chunk, kbt) units.

    scores/exp of unit i run while attnV of unit i-1 executes -- including
    across chunk boundaries, so the ScalarE exp stream never waits for a
    chunk's attnV tail.  Deferred work (x-DMA, projections, norm/out of
    finished chunks) is queued with a deadline (the unit whose scores need
    it) and drained one item per unit into the PE/DVE slack.
    """
    nqc, kbc, f32 = env["nqc"], env["kbc"], env["f32"]
    units = [(qc, kbt) for qc in range(nqc) for kbt in range(2 * (qc + 1))]
    uidx = {u: i for i, u in enumerate(units)}
    INF = len(units) + 1  # deadline index meaning 'whenever'

    _dma_x(env, 0)
    _dma_x(env, 1)
    _proj_qk(env, 0)

    todo = []  # FIFO of ((deadline_unit_index, phase), emit_fn)
    # phase 0: must be emitted before that unit's scores (qk/x-DMA feed them)
    # phase 1: before that unit's attnV (v-projection feeds it)

    def pop_due(key):
        # FIFO-drain until no queued item is past due (front items with a
        # later deadline are emitted early rather than reordered)
        while any(dl <= key for dl, _ in todo):
            todo.pop(0)[1]()

    def pop_one():
        if todo:
            todo.pop(0)[1]()

    acc = {}

    def flush_attnv(pqc, pkbt, ex):
        if pkbt == 0:
            acc[pqc] = [env["accps"].tile([65, QCHUNK], f32,
                                          name=f"acc{h}_{pqc}_{rep}",
                                          tag=f"ac{h}") for h in range(2)]
        nkb = (pqc + 1) * kbc
        _attnv(env, pqc, pkbt, acc[pqc], ex, nkb)
        if pkbt == nkb // 2 - 1:  # chunk done: evacuate, queue norm/out
            su = [None, None]
            _acc_evac(env, pqc, acc.pop(pqc), su)
            last = pqc == nqc - 1
            todo.append(((INF, 0), lambda q=pqc, s=su: _norm_out(env, q, sums=s)))
            for b in range(0, kbc, 2):
                todo.append(((INF, 0), lambda q=pqc, b=b, la=last:
                             _norm_out(env, q, blocks=(b, b + 1), last=la)))

    sc = [None, None]
    pend = None
    for i, (qc, kbt) in enumerate(units):
        if kbt == 0:
            if qc == 0:
                todo.append(((uidx[(0, 1)], 1), lambda: _proj_v(env, 0)))
                if nqc > 1:
                    todo.append(((uidx[(1, 0)], 0), lambda: _proj_qk(env, 1)))
                    todo.append(((uidx[(1, 1)], 1), lambda: _proj_v(env, 1)))
            if qc + 2 < nqc:
                q2 = qc + 2
                todo.append(((uidx[(q2, 0)], 0), lambda q=q2: _dma_x(env, q)))
                todo.append(((uidx[(q2, 0)], 0), lambda q=q2: _proj_qk(env, q)))
                todo.append(((uidx[(q2, 1)], 1), lambda q=q2: _proj_v(env, q, 0)))
                todo.append(((uidx[(q2, 1)], 1), lambda q=q2: _proj_v(env, q, 1)))
        pop_due((i, 0))
        ex = _scores(env, qc, kbt, sc)
        pop_one()
        pop_due((i, 1))
        if pend is not None:
            flush_attnv(*pend)
        pend = (qc, kbt, ex)
    flush_attnv(*pend)
    while todo:
        todo.pop(0)[1]()


def _in_maps(x, W_q, W_k, W_v, W_proj):
    maps = []
    for c in range(8):
        b, hp = c // 4, c % 4
        cols = slice(hp * 128, (hp + 1) * 128)
        maps.append({
            "xT": np.ascontiguousarray(x[b].T),
            "wqT": np.ascontiguousarray(W_q.T[:, cols]),
            "wkT": np.ascontiguousarray(W_k.T[:, cols]),
            "wvT": np.ascontiguousarray(W_v.T[:, cols]),
            "wpT": np.ascontiguousarray(W_proj[:, cols].T),
            "ones_in": np.ones((128, 64), dtype=np.float32),
        })
    return maps


def kernel(x, W_q, W_k, W_v, W_proj, b_proj, _trace=False):
    from concourse.bass_utils import run_bass_kernel_spmd

    x = np.asarray(x, dtype=np.float32)
    W_q = np.asarray(W_q, dtype=np.float32)
    W_k = np.asarray(W_k, dtype=np.float32)
    W_v = np.asarray(W_v, dtype=np.float32)
    W_proj = np.asarray(W_proj, dtype=np.float32)
    b_proj = np.asarray(b_proj, dtype=np.float32)

    if "nc" not in _CACHE:
        _CACHE["nc"] = _build()
    nc = _CACHE["nc"]

    res = run_bass_kernel_spmd(nc, _in_maps(x, W_q, W_k, W_v, W_proj),
                               core_ids=list(range(8)), trace=_trace)
    out = np.empty((B, S, D), dtype=np.float32)
    for b in range(B):
        acc = res.results[4 * b]["out_p"].astype(np.float32)
        for j in range(1, 4):
            acc = acc + res.results[4 * b + j]["out_p"]
        out[b] = acc + b_proj
    if _trace:
        _CACHE["last_trace"] = res
    return out
